# revision 1
# baseline (speedup 1.0000x reference)
"""EvolveGCN-reg Trainium2 kernel (8 NeuronCores, timestep-parallel).

Math: the reference computes, per timestep t (scan carrying a GRU-evolved
16x16 weight W):
    y_t   = X_t @ p / ||p||;  (yk, idx) = top16(y_t);  Xs = (X_t[idx] * yk).T
    W_t   = GRU(W_{t-1}, Xs)          (16x16 matmuls, tiny)
    AH    = segment_sum(val * X_t[col], row, N)       (3.2M-edge sparse op)
    out_t = (AH @ W_t) @ lin_w + b
Key identity used here:  out_t = A_t @ (X_t @ (W_t @ lin_w)) + b, so the
feature dimension collapses and the sparse phase is a scalar gather /
segment-sum:  out_t[n] = b + sum_{e: row[e]=n} val[e] * s_t[col[e]],
with s_t = X_t @ u_t and u_t = W_t @ lin_w.

Sharding: core t owns timestep t (uniform load, no collectives). Host does
index-space layout only (the sharding hint's "partition edge lists by
destination range" taken to its conclusion): edges are grouped by
destination, destinations degree-sorted and assigned round-robin to the 128
SBUF partitions so each "rank" of 128 nodes shares a common padded segment
length. The segment-sum then becomes ~45 strided DVE tensor_reduce ops at
line rate. The one index permutation (s gathered edge-wise) happens during
host-side re-staging between device launches; every floating-point
operation of the model runs on the NeuronCores.

Device launches (all math on device):
  L1: yraw_t = X_t @ p                     -> host extracts top-16 *indices*
  L2: ||p|| normalize, Xs_tau, full GRU chain, u_t select (one-hot input
      mask), s_t = X_t @ u_t
  L3: w = val * s[col] (gathered layout), per-rank segmented reduce, + b
"""

import numpy as np
from contextlib import ExitStack

import concourse.bass as bass
import concourse.bacc as bacc
import concourse.tile as tile
from concourse import mybir
from concourse.bass_utils import run_bass_kernel_spmd

dt = mybir.dt

T, N, E, F0, F1 = 8, 100000, 3200000, 16, 16
NCORES = 8
P = 128
RANKS = (N + P - 1) // P  # 782
N_PAD = P * RANKS  # 100096
CORE_IDS = list(range(NCORES))

_cache = {}


def _axon_reset():
    try:
        import ctypes

        lib = ctypes.CDLL("/opt/axon/libaxon_pjrt.so")
        lib.axon_reset.restype = ctypes.c_int64
        lib.axon_reset()
    except Exception:
        pass


def _run(nc, in_maps):
    try:
        return run_bass_kernel_spmd(nc, in_maps, core_ids=CORE_IDS)
    except Exception:
        _axon_reset()
        return run_bass_kernel_spmd(nc, in_maps, core_ids=CORE_IDS)

GP_FEATS = (4, 5, 6, 7)  # features accumulated on GpSimd (rest on DVE)


def _emit_matvec(nc, io, acc_pool, xt_ap, u_sb, out_sb):
    """out_sb[p, r] = sum_f X_T[p, f, r] * u_sb[p, f]; X_T streamed in 8
    feature-chunks, fused multiply-accumulate chain on DVE."""
    xt = io.tile([P, F0, RANKS], dt.float32, tag="xt", name="xt")
    FC = F0 // 8
    for c4 in range(8):
        nc.sync.dma_start(
            xt[:, c4 * FC : (c4 + 1) * FC, :],
            xt_ap[:, c4 * FC * RANKS : (c4 + 1) * FC * RANKS].rearrange(
                "p (f r) -> p f r", r=RANKS),
        )
    # two interleaved accumulator chains hide DVE issue gaps
    acc_a = acc_pool.tile([P, RANKS], dt.float32, tag="acc_a", name="acc_a")
    acc_b = acc_pool.tile([P, RANKS], dt.float32, tag="acc_b", name="acc_b")
    nc.vector.tensor_scalar_mul(acc_a[:], xt[:, 0, :], u_sb[:, 0:1])
    nc.vector.tensor_scalar_mul(acc_b[:], xt[:, 1, :], u_sb[:, 1:2])
    for f in range(2, F0):
        acc = acc_a if f % 2 == 0 else acc_b
        nc.vector.scalar_tensor_tensor(
            out=acc[:], in0=xt[:, f, :], scalar=u_sb[:, f : f + 1],
            in1=acc[:], op0=mybir.AluOpType.mult, op1=mybir.AluOpType.add,
        )
    nc.vector.tensor_tensor(out=out_sb[:], in0=acc_a[:], in1=acc_b[:],
                            op=mybir.AluOpType.add)


# ---------------------------------------------------------------- launch 1
def _build_p1():
    nc = bacc.Bacc("TRN2", target_bir_lowering=False, debug=False)
    xt_ap = nc.dram_tensor("XT", [P, F0 * RANKS], dt.float32, kind="ExternalInput").ap()
    prep_ap = nc.dram_tensor("prep", [P, F0], dt.float32, kind="ExternalInput").ap()
    y_ap = nc.dram_tensor("yraw", [P, RANKS], dt.float32, kind="ExternalOutput").ap()

    with tile.TileContext(nc) as tc, ExitStack() as ctx:
        io = ctx.enter_context(tc.tile_pool(name="io", bufs=1))
        yp = ctx.enter_context(tc.tile_pool(name="y", bufs=1))
        p_t = yp.tile([P, F0], dt.float32)
        nc.scalar.dma_start(p_t[:], prep_ap[:])
        y_t = yp.tile([P, RANKS], dt.float32)
        _emit_matvec(nc, io, yp, xt_ap, p_t, y_t)
        nc.sync.dma_start(y_ap[:], y_t[:])
    nc.compile()
    return nc


# ---------------------------------------------------------------- launch 2
# packed small-input column layout (one [16, 352] DMA): see _pack_smalls
_COLS = {}
_off = 0
for _n, _w in [("X16", 128), ("yraw16", 8), ("WZT", 16), ("UZT", 16), ("BZT", 16),
               ("WRT", 16), ("URT", 16), ("BRT", 16), ("WHT", 16), ("UHT", 16),
               ("BHT", 16), ("Winit", 16), ("I16", 16), ("linw_rep", 16),
               ("sel", 8), ("prep16", 16)]:
    _COLS[_n] = (_off, _off + _w)
    _off += _w
SMALLS_W = _off


def _build_p2():
    nc = bacc.Bacc("TRN2", target_bir_lowering=False, debug=False)
    xt_ap = nc.dram_tensor("XT", [P, F0 * RANKS], dt.float32, kind="ExternalInput").ap()
    sm_ap = nc.dram_tensor("smalls", [16, SMALLS_W], dt.float32, kind="ExternalInput").ap()
    s_ap = nc.dram_tensor("s", [P, RANKS], dt.float32, kind="ExternalOutput").ap()

    with tile.TileContext(nc) as tc, ExitStack() as ctx:
        small = ctx.enter_context(tc.tile_pool(name="small", bufs=1))
        gru = ctx.enter_context(tc.tile_pool(name="gru", bufs=2))
        ps = ctx.enter_context(tc.tile_pool(name="ps", bufs=2, space="PSUM"))
        psxs = ctx.enter_context(tc.tile_pool(name="psxs", bufs=2, space="PSUM"))
        io = ctx.enter_context(tc.tile_pool(name="io", bufs=1))
        sp = ctx.enter_context(tc.tile_pool(name="s", bufs=1))

        sm = small.tile([16, SMALLS_W], dt.float32)
        nc.scalar.dma_start(sm[:], sm_ap[:])

        # prefetch ACT function tables while the DMA is in flight
        warm = small.tile([1, 2], dt.float32)
        nc.vector.memset(warm[:], 0.0)
        nc.scalar.activation(warm[:, 0:1], warm[:, 0:1],
                             mybir.ActivationFunctionType.Sigmoid)
        nc.scalar.activation(warm[:, 1:2], warm[:, 1:2],
                             mybir.ActivationFunctionType.Tanh)

        def gi(name):
            a, b = _COLS[name]
            return sm[:, a:b]

        # invp = 1/||p|| on partition 0, replicated to 16 partitions via PE
        psq = small.tile([1, F0], dt.float32)
        nc.scalar.square(psq[:], gi("prep16")[0:1, :])
        pss = small.tile([1, 1], dt.float32)
        nc.vector.tensor_reduce(out=pss[:], in_=psq[:], axis=mybir.AxisListType.X,
                                op=mybir.AluOpType.add)
        pnorm = small.tile([1, 1], dt.float32)
        nc.scalar.sqrt(pnorm[:], pss[:])
        invp = small.tile([1, 1], dt.float32)
        nc.vector.reciprocal(invp[:], pnorm[:])
        ones1x16 = small.tile([1, 16], dt.float32)
        nc.vector.memset(ones1x16[:], 1.0)
        invp16_ps = ps.tile([16, 1], dt.float32, tag="misc", name="invp16_ps")
        nc.tensor.matmul(invp16_ps[:], ones1x16[:], invp[:], start=True, stop=True)
        invp16 = small.tile([16, 1], dt.float32)
        nc.scalar.copy(invp16[:], invp16_ps[:])

        # Xs_tau = (X16_tau^T @ diag(yraw_tau)) * invp   (scale fused in copy)
        Xs = []
        for tau in range(T):
            dg = gru.tile([16, 16], dt.float32, tag="diag", name=f"dg{tau}")
            nc.vector.tensor_scalar_mul(dg[:], gi("I16"),
                                        gi("yraw16")[:, tau : tau + 1])
            xs_ps = psxs.tile([16, 16], dt.float32, tag="xs", name=f"xsps{tau}")
            nc.tensor.matmul(xs_ps[:], gi("X16")[:, tau * F0 : (tau + 1) * F0],
                             dg[:], start=True, stop=True)
            xs = gru.tile([16, 16], dt.float32, tag="xs_sb", name=f"xs{tau}")
            nc.vector.tensor_scalar_mul(xs[:], xs_ps[:], invp16[:])
            Xs.append(xs)

        # GRU chain; bias folded into the PE accumulation group
        u_cols = small.tile([16, T], dt.float32)
        W = gi("Winit")
        for tau in range(T):
            def gate(wt, ut, bt, rhs2, func, tag):
                acc = ps.tile([16, 16], dt.float32, tag="mm", name=f"mm{tag}{tau}")
                nc.tensor.matmul(acc[:], gi(wt), Xs[tau][:], start=True, stop=False)
                nc.tensor.matmul(acc[:], gi(bt), gi("I16"), start=False, stop=False)
                nc.tensor.matmul(acc[:], gi(ut), rhs2[:], start=False, stop=True)
                g = gru.tile([16, 16], dt.float32, tag=f"g{tag}", name=f"g{tag}{tau}")
                nc.scalar.activation(g[:], acc[:], func)
                return g

            Zg = gate("WZT", "UZT", "BZT", W, mybir.ActivationFunctionType.Sigmoid, "z")
            Rg = gate("WRT", "URT", "BRT", W, mybir.ActivationFunctionType.Sigmoid, "r")
            RW = gru.tile([16, 16], dt.float32, tag="rw", name=f"rw{tau}")
            nc.vector.tensor_tensor(out=RW[:], in0=Rg[:], in1=W[:],
                                    op=mybir.AluOpType.mult)
            Ht = gate("WHT", "UHT", "BHT", RW, mybir.ActivationFunctionType.Tanh, "h")

            HmW = gru.tile([16, 16], dt.float32, tag="hmw", name=f"hmw{tau}")
            nc.vector.tensor_tensor(out=HmW[:], in0=Ht[:], in1=W[:],
                                    op=mybir.AluOpType.subtract)
            ZH = gru.tile([16, 16], dt.float32, tag="zh", name=f"zh{tau}")
            nc.vector.tensor_tensor(out=ZH[:], in0=Zg[:], in1=HmW[:],
                                    op=mybir.AluOpType.mult)
            Wn = gru.tile([16, 16], dt.float32, tag=f"w{tau}", name=f"w{tau}")
            nc.vector.tensor_tensor(out=Wn[:], in0=W[:], in1=ZH[:],
                                    op=mybir.AluOpType.add)
            W = Wn

            um = gru.tile([16, 16], dt.float32, tag="um", name=f"um{tau}")
            nc.vector.tensor_tensor(out=um[:], in0=W[:], in1=gi("linw_rep"),
                                    op=mybir.AluOpType.mult)
            nc.vector.tensor_reduce(out=u_cols[:, tau : tau + 1], in_=um[:],
                                    axis=mybir.AxisListType.X, op=mybir.AluOpType.add)

        # select this core's u via one-hot input mask; broadcast to 128 parts
        usm = small.tile([16, T], dt.float32)
        nc.vector.tensor_tensor(out=usm[:], in0=u_cols[:], in1=gi("sel"),
                                op=mybir.AluOpType.mult)
        u_sel = small.tile([16, 1], dt.float32)
        nc.vector.tensor_reduce(out=u_sel[:], in_=usm[:], axis=mybir.AxisListType.X,
                                op=mybir.AluOpType.add)
        diag_u = small.tile([16, 16], dt.float32)
        nc.vector.tensor_scalar_mul(diag_u[:], gi("I16"), u_sel[:])
        ones16x128 = small.tile([16, P], dt.float32)
        nc.vector.memset(ones16x128[:], 1.0)
        ub_ps = ps.tile([P, 16], dt.float32, tag="misc", name="ub_ps")
        nc.tensor.matmul(ub_ps[:], ones16x128[:], diag_u[:], start=True, stop=True)
        ub = small.tile([P, 16], dt.float32)
        nc.scalar.copy(ub[:], ub_ps[:])

        s_t = sp.tile([P, RANKS], dt.float32)
        _emit_matvec(nc, io, sp, xt_ap, ub, s_t)
        nc.sync.dma_start(s_ap[:], s_t[:])
    nc.compile()
    return nc


# ---------------------------------------------------------------- launch 3
def _build_p3(Ls, chunks, f_pad, bf16=False):
    nc = bacc.Bacc("TRN2", target_bir_lowering=False, debug=False)
    in_dt = dt.bfloat16 if bf16 else dt.float32
    tot = sum(sum(L * cnt for (L, cnt, _) in runs) for _, runs in chunks) * P
    sg_ap = nc.dram_tensor("sg", [tot], in_dt, kind="ExternalInput").ap()
    val_ap = nc.dram_tensor("val", [tot], in_dt, kind="ExternalInput").ap()
    b_ap = nc.dram_tensor("linb", [P, 1], dt.float32, kind="ExternalInput").ap()
    y_ap = nc.dram_tensor("y", [P, RANKS], dt.float32, kind="ExternalOutput").ap()

    with tile.TileContext(nc) as tc, ExitStack() as ctx:
        io = ctx.enter_context(tc.tile_pool(name="io", bufs=3))
        yp = ctx.enter_context(tc.tile_pool(name="y", bufs=1))
        b_t = yp.tile([P, 1], dt.float32)
        nc.scalar.dma_start(b_t[:], b_ap[:])
        y_t = yp.tile([P, RANKS], dt.float32)
        for ci, (col0, runs) in enumerate(chunks):
            ncols = sum(L * cnt for (L, cnt, _) in runs)
            sg_t = io.tile([P, ncols], in_dt, tag="sg", name="sg_t")
            nc.sync.dma_start(
                sg_t[:], sg_ap[col0 * P : (col0 + ncols) * P].rearrange(
                    "(p j) -> p j", j=ncols))
            val_t = io.tile([P, ncols], in_dt, tag="val", name="val_t")
            nc.sync.dma_start(
                val_t[:], val_ap[col0 * P : (col0 + ncols) * P].rearrange(
                    "(p j) -> p j", j=ncols))
            w_t = io.tile([P, ncols], dt.float32, tag="w", name="w_t")
            nc.vector.tensor_tensor(out=w_t[:], in0=sg_t[:], in1=val_t[:],
                                    op=mybir.AluOpType.mult)
            c = 0
            for L, cnt, rank0 in runs:
                seg = w_t[:, c : c + cnt * L].rearrange("p (r l) -> p r l", l=L)
                nc.vector.tensor_reduce(
                    out=y_t[:, rank0 : rank0 + cnt], in_=seg,
                    axis=mybir.AxisListType.X, op=mybir.AluOpType.add,
                )
                c += cnt * L
        yb = yp.tile([P, RANKS], dt.float32)
        nc.vector.tensor_scalar_add(yb[:], y_t[:], b_t[:])
        nc.sync.dma_start(y_ap[:], yb[:])
    nc.compile()
    return nc


# ------------------------------------------------------------ host layout
def _edge_layout(edge_row, edge_col, edge_val):
    """Degree-sorted, rank-equalized destination layout shared across T."""
    degs = np.zeros((T, N_PAD), np.int64)
    orders = np.zeros((T, N_PAD), np.int64)
    for t in range(T):
        deg = np.bincount(edge_row[t].astype(np.int64), minlength=N_PAD)
        degs[t] = deg
        orders[t] = np.argsort(-deg, kind="stable")
    rank_max = np.zeros((T, RANKS), np.int64)
    for t in range(T):
        rank_max[t] = degs[t][orders[t]].reshape(RANKS, P).max(1)
    Ls = rank_max.max(0)
    Ls = np.maximum.accumulate(Ls[::-1])[::-1]  # enforce non-increasing
    Ls = np.maximum(Ls, 1)
    offs = np.zeros(RANKS + 1, np.int64)
    offs[1:] = np.cumsum(Ls)
    f_pad = int(-(-offs[-1] // 8) * 8)

    col_layout = np.zeros((T, P, f_pad), np.int32)
    val_layout = np.zeros((T, P, f_pad), np.float32)
    for t in range(T):
        row = edge_row[t].astype(np.int64)
        order = orders[t]
        slot_of_node = np.empty(N_PAD, np.int64)
        slot_of_node[order] = np.arange(N_PAD)
        ord_e = np.argsort(row, kind="stable")
        rows_s = row[ord_e]
        deg = degs[t]
        node_start = np.zeros(N_PAD, np.int64)
        node_start[1:] = np.cumsum(deg)[:-1]
        k = np.arange(E, dtype=np.int64) - node_start[rows_s]
        s = slot_of_node[rows_s]
        p_idx = s % P
        r_idx = s // P
        pos = offs[r_idx] + k
        col_layout[t, p_idx, pos] = edge_col[t][ord_e]
        val_layout[t, p_idx, pos] = edge_val[t][ord_e]

    # chunk schedule shared across cores
    FC = 3400
    chunks = []
    cur, cur_cols, col0, r = [], 0, 0, 0
    while r < RANKS:
        L = int(Ls[r])
        cnt = 0
        while r + cnt < RANKS and Ls[r + cnt] == L and cur_cols + (cnt + 1) * L <= FC:
            cnt += 1
        if cnt == 0:
            chunks.append((col0, cur))
            col0 += cur_cols
            cur, cur_cols = [], 0
            continue
        cur.append((L, cnt, r))
        cur_cols += cnt * L
        r += cnt
    if cur:
        chunks.append((col0, cur))
    return Ls, offs, f_pad, col_layout, val_layout, orders, chunks


# ------------------------------------------------------------------ kernel
def kernel(**inputs):
    inp = {k: np.asarray(v) for k, v in inputs.items()}
    X = inp["X"].astype(np.float32, copy=False)  # [T, N, F0]
    edge_row = inp["edge_row"]
    edge_col = inp["edge_col"]
    edge_val = inp["edge_val"].astype(np.float32, copy=False)
    p = inp["p"].astype(np.float32, copy=False)

    # padded, partition-major, feature-transposed X per core:
    # node n = p*RANKS + i;  XT[core t][p, f*RANKS + i] = X[t, n, f]
    X_pad = np.zeros((T, N_PAD, F0), np.float32)
    X_pad[:, :N] = X
    XT_core = np.ascontiguousarray(
        X_pad.reshape(T, P, RANKS, F0).transpose(0, 1, 3, 2)
    ).reshape(T, P, F0 * RANKS)

    Ls, offs, f_pad, col_layout, val_layout, orders, chunks = _edge_layout(
        edge_row, edge_col, edge_val
    )

    # ---- launch 1: yraw_t = X_t @ p
    if "p1" not in _cache:
        _cache["p1"] = _build_p1()
    p_rep = np.tile(p[None, :], (P, 1))
    in1 = [{"XT": XT_core[t], "prep": p_rep} for t in range(T)]
    res1 = _run(_cache["p1"], in1)
    yraw = np.stack([res1.results[t]["yraw"].reshape(-1) for t in range(T)])

    # ---- host: top-16 indices (index selection only)
    yraw16 = np.zeros((16, T), np.float32)
    X16 = np.zeros((16, T * F0), np.float32)
    for t in range(T):
        y = yraw[t][:N]
        cand = np.argpartition(y, -32)[-32:]
        order = cand[np.lexsort((cand, -y[cand]))][:16]
        yraw16[:, t] = y[order]
        X16[:, t * F0 : (t + 1) * F0] = X[t][order]

    # ---- launch 2: GRU chain + s_t = X_t @ (W_t @ lin_w)
    if "p2" not in _cache:
        _cache["p2"] = _build_p2()
    f32 = np.float32
    smalls = np.zeros((16, SMALLS_W), f32)

    def put(name, arr):
        a, b = _COLS[name]
        smalls[:, a:b] = arr

    put("X16", X16)
    put("yraw16", yraw16)
    put("WZT", inp["W_Z"].T.astype(f32))
    put("UZT", inp["U_Z"].T.astype(f32))
    put("BZT", inp["B_Z"].T.astype(f32))
    put("WRT", inp["W_R"].T.astype(f32))
    put("URT", inp["U_R"].T.astype(f32))
    put("BRT", inp["B_R"].T.astype(f32))
    put("WHT", inp["W_H"].T.astype(f32))
    put("UHT", inp["U_H"].T.astype(f32))
    put("BHT", inp["B_H"].T.astype(f32))
    put("Winit", inp["W_init"].astype(f32))
    put("I16", np.eye(16, dtype=f32))
    put("linw_rep", np.tile(inp["lin_w"].astype(f32)[None, :], (16, 1)))
    put("prep16", np.tile(p[None, :], (16, 1)))
    in2 = []
    for t in range(T):
        sm_t = smalls.copy()
        sel = np.zeros((16, T), f32)
        sel[:, t] = 1.0
        a, b = _COLS["sel"]
        sm_t[:, a:b] = sel
        in2.append({"XT": XT_core[t], "smalls": sm_t})
    res2 = _run(_cache["p2"], in2)
    s_all = np.stack([res2.results[t]["s"].reshape(-1) for t in range(T)])

    # ---- host re-staging: gather s into the edge layout (index move only),
    # flattened chunk-major so every L3 DMA chunk is one contiguous block
    def _chunk_flat(arr2d):
        return np.concatenate(
            [arr2d[:, c0 : c0 + sum(L * n for (L, n, _) in runs)].reshape(-1)
             for c0, runs in chunks])

    sg = np.empty((T, P, f_pad), np.float32)
    for t in range(T):
        sg[t] = s_all[t][col_layout[t]]
    sgf = [_chunk_flat(sg[t]) for t in range(T)]
    valf = [_chunk_flat(val_layout[t]) for t in range(T)]

    # ---- launch 3: w = val*sg, segmented reduce per rank, + lin_b
    key3 = ("p3", f_pad, tuple(Ls.tolist()))
    if key3 not in _cache:
        _cache[key3] = _build_p3(Ls, chunks, f_pad)
    b_rep = np.full((P, 1), np.float32(inp["lin_b"][0]), np.float32)
    in3 = [{"sg": sgf[t], "val": valf[t], "linb": b_rep} for t in range(T)]
    res3 = _run(_cache[key3], in3)

    # ---- host: un-permute ranks back to node ids
    out = np.zeros((T, N), np.float32)
    for t in range(T):
        y3 = res3.results[t]["y"]  # [P, RANKS]; slot s=128r+p -> y3[p, r]
        flat = np.ascontiguousarray(y3.T).reshape(-1)
        full = np.empty(N_PAD, np.float32)
        full[orders[t]] = flat
        out[t] = full[:N]
    return out

